# revision 3
# baseline (speedup 1.0000x reference)
"""Gate-major tanh-only Trainium2 kernel for nn_DetectorHelper (seq2seq LSTM).

v2: single PSUM bank per 4-step group holds all 8 gate chunks; one
tanh(0.5*gates) ACT op per step produces tf,ti,tg,to (sigmoid via
(tanh(x/2)+1)/2 with gate-row scales folded into host weights; g rows
doubled so tanh(0.5*2g)=tanh(g)). Cell math is 3 fused
scalar_tensor_tensor DVE ops on state D=2c:
    A = (tf+1)*D_prev        B = (ti+1)*tg        D = 0.5*A + B
    th = tanh(0.5*D)  [ACT]  h2 = (to+1)*th  (= 2h; W_hh/W_out cols
pre-scaled by 0.5 on host).

Decoder projection batched per 32 steps from the bf16 h2 history;
b_out, time reversal and transpose applied on host at gather.
"""

import sys

sys.path.insert(0, "/opt/trn_rl_repo")

from contextlib import ExitStack

import numpy as np

B = 16      # batch rows per core
F = 64      # feature dim
H = 256     # hidden dim
G = 4 * H   # gate dim
NCH = 8     # gate chunks of 128
GRP = 4     # steps per PSUM group (8 chunks x 4 x 16 = 512 = one bank)
SG = 8      # steps per x staging DMA
N_CORES = 8

_CACHE = {}


def _build(repeat=1, external_io=True, num_devices=N_CORES, T=1024, U=128,
           static_loops=False, wdt_name="bfloat16", dbg=False,
           loop_hints=False, stag=False, unroll=1):
    import concourse.bass as bass
    import concourse.tile as tile
    from concourse import bacc, mybir

    F32 = mybir.dt.float32
    BF16 = mybir.dt.bfloat16
    WDT = getattr(mybir.dt, wdt_name)
    NB = T // U
    NPG = U // GRP
    NSG = U // SG

    nc = bacc.Bacc("TRN2", target_bir_lowering=False, debug=False,
                   num_devices=num_devices)

    KI = "ExternalInput" if external_io else "Internal"
    KO = "ExternalOutput" if external_io else "Internal"
    XPAD = 2 * SG * B
    xte_d = nc.dram_tensor("xte", [F + 1, T * B + XPAD], F32, kind=KI).ap()
    xtd_d = nc.dram_tensor("xtd", [F + 1, T * B + XPAD], F32, kind=KI).ap()
    whh_e_d = nc.dram_tensor("whh_e", [128, 16 * 128], F32, kind=KI).ap()
    wih_e_d = nc.dram_tensor("wih_e", [128, 8 * 128], F32, kind=KI).ap()
    whh_d_d = nc.dram_tensor("whh_d", [128, 16 * 128], F32, kind=KI).ap()
    wih_d_d = nc.dram_tensor("wih_d", [128, 8 * 128], F32, kind=KI).ap()
    wout_d = nc.dram_tensor("wout", [128, 2 * F], F32, kind=KI).ap()
    out_d = nc.dram_tensor("out", [F, T * B], F32, kind=KO).ap()
    sink_d = None
    if not external_io:
        sink_d = nc.dram_tensor("sink", [1, 4], F32,
                                kind="ExternalOutput").ap()
    dbg_t_d = dbg_g_d = dbg_h_d = None
    if dbg:
        NDBG = 8
        dbg_t_d = nc.dram_tensor("dbg_t", [128, NDBG * 128], F32,
                                 kind="ExternalOutput").ap()
        dbg_g_d = nc.dram_tensor("dbg_g", [128, NDBG * 128], F32,
                                 kind="ExternalOutput").ap()
        dbg_h_d = nc.dram_tensor("dbg_h", [128, 2 * 65 * B], F32,
                                 kind="ExternalOutput").ap()

    with tile.TileContext(nc) as tc, ExitStack() as ctx:
        wpool = ctx.enter_context(tc.tile_pool(name="wpool", bufs=1))
        whh_e = wpool.tile([128, 16 * 128], WDT, name="whh_e_sb")
        wih_e = wpool.tile([128, 8 * 128], WDT, name="wih_e_sb")
        whh_d = wpool.tile([128, 16 * 128], WDT, name="whh_d_sb")
        wih_d = wpool.tile([128, 8 * 128], WDT, name="wih_d_sb")
        wout = wpool.tile([128, 2 * F], BF16, name="wout_sb")
        for sb, dr in [(whh_e, whh_e_d), (wih_e, wih_e_d), (whh_d, whh_d_d),
                       (wih_d, wih_d_d), (wout, wout_d)]:
            stg = wpool.tile(list(sb.shape), F32, name="wstg", tag="wstg",
                             bufs=2)
            nc.sync.dma_start(stg[:], dr[:])
            nc.vector.tensor_copy(sb[:], stg[:])

        # persistent state: h2 history per k-chunk; X holds the D=2c
        # ping-pong co-located with the per-step tanh outputs so that
        # [D_prev | tg] and [tf | ti] are contiguous 64-col slices:
        # X = [D_p0 | tg tf ti to (p0) | D_p1 | tg tf ti to (p1)]
        HREG = (U + 1) * B
        hhist = wpool.tile([128, 2 * HREG], BF16, name="hhist_sb")
        xdt = wpool.tile([128, 320], F32, name="xdt_sb")

        xstage = wpool.tile([128, 2 * SG * B], F32, name="xstage_sb")
        xstager = wpool.tile([128, 2 * SG * B], BF16, name="xstager_sb")

        g_pool = ctx.enter_context(
            tc.tile_pool(name="g_pool", bufs=2, space="PSUM"))
        pr_pool = ctx.enter_context(
            tc.tile_pool(name="pr_pool", bufs=2, space="PSUM"))
        apool = ctx.enter_context(tc.tile_pool(name="apool", bufs=3))
        spool = ctx.enter_context(tc.tile_pool(name="spool", bufs=2))

        TANH = mybir.ActivationFunctionType.Tanh
        ADD = mybir.AluOpType.add
        MULT = mybir.AluOpType.mult

        hview = hhist[:].rearrange("p (k s n) -> p k s n", k=2, s=U + 1, n=B)

        def init_state():
            nc.vector.memset(hhist[:, 0:B], 0.0)
            nc.vector.memset(hhist[:, HREG:HREG + B], 0.0)
            nc.vector.memset(xdt[:, 0:32], 0.0)
            nc.vector.memset(xstager[:], 0.0)

        def hslot(s, k):
            return hhist[:, k * HREG + s * B: k * HREG + s * B + B]

        def stage8(blk, s, xsrc_d):
            half = (s % 2) * SG * B
            nc.sync.dma_start(xstage[0:F + 1, half:half + SG * B],
                              xsrc_d[:, bass.ts(blk * NSG + s, SG * B)])
            nc.gpsimd.tensor_copy(xstager[0:F + 1, half:half + SG * B],
                                  xstage[0:F + 1, half:half + SG * B])

        def x_group_mm(c, g, wih, g_ps):
            """x-part+bias MM for gate chunk c of psum-group g (N=64).

            start=True clears has_written for the WHOLE bank, so only the
            first chunk's MM may carry it; later chunks write fresh
            (has_written=0 after the clear) with start=False, and the
            per-step h-MMs then accumulate."""
            off = 128 * ((g // 2) % 2) + 64 * (g % 2)
            nc.tensor.matmul(g_ps[:, c * 64:(c + 1) * 64],
                             wih[:, c * 128:(c + 1) * 128],
                             xstager[:, off:off + GRP * B],
                             start=(c == 0), stop=True)

        def step(j, whh, wih, g_ps, nxt_ps, dec=None, dbg_sb=None):
            jj = j % GRP
            p = j % 2
            q = 1 - p

            if nxt_ps is not None:
                # two x-MMs of the NEXT group; deps met a step ago, so the
                # in-order PE drains them while waiting for this step's h
                for c in (2 * jj, 2 * jj + 1):
                    x_group_mm(c, j // GRP + 1, wih, nxt_ps)

            for c in range(NCH):
                o_ap = g_ps[:, c * 64 + jj * B: c * 64 + jj * B + B]
                for k in (0, 1):
                    nc.tensor.matmul(
                        o_ap,
                        whh[:, (2 * c + k) * 128:(2 * c + k + 1) * 128],
                        hslot(j, k), start=False, stop=(k == 1))

            if dec is not None and j % 32 == 31:
                ostage, = dec
                hh = j // 32
                o_ps = pr_pool.tile([F, 512], F32, name="o_ps")
                for k in (0, 1):
                    nc.tensor.matmul(
                        o_ps[:], wout[:, k * F:(k + 1) * F],
                        hhist[:, k * HREG + (j - 31) * B:
                              k * HREG + (j + 1) * B],
                        start=(k == 0), stop=(k == 1))
                nc.vector.tensor_copy(ostage[:, hh * 512:(hh + 1) * 512],
                                      o_ps[:])

            # cell math: tanh(0.5*gates) split: [g f i] feeds the chain,
            # [o] is only needed by h2 much later (chunk order g f i o)
            t_sb = xdt[:, 160 * p + 32:160 * p + 160]
            g_in = g_ps[:].rearrange(
                "p (c n) -> p c n", c=NCH, n=64)[:, :, jj * B:(jj + 1) * B]
            nc.scalar.activation(
                xdt[:, 160 * p + 32:160 * p + 128].rearrange(
                    "p (c n) -> p c n", c=6, n=B),
                g_in[:, 0:6], TANH, scale=0.5)
            nc.scalar.activation(
                xdt[:, 160 * p + 128:160 * p + 160].rearrange(
                    "p (c n) -> p c n", c=2, n=B),
                g_in[:, 6:8], TANH, scale=0.5)
            if dbg_sb is not None and j < 8:
                dt_sb, dg_sb = dbg_sb
                nc.vector.tensor_copy(dt_sb[:, j * 128:(j + 1) * 128],
                                      t_sb)
                nc.vector.tensor_copy(
                    dg_sb[:, j * 128:(j + 1) * 128].rearrange(
                        "p (c n) -> p c n", c=NCH, n=B), g_in)
            to = xdt[:, 160 * p + 128:160 * p + 160]
            ab = apool.tile([128, 64], F32, name="ab")
            tch = apool.tile([128, 32], F32, name="tch")
            # [A|B] = ([tf|ti] + 1) * [D_prev|tg] in one wide stt
            nc.vector.scalar_tensor_tensor(
                ab[:], xdt[:, 160 * p + 64:160 * p + 128], 1.0,
                xdt[:, 160 * p:160 * p + 64], ADD, MULT)
            nc.vector.scalar_tensor_tensor(
                xdt[:, 160 * q:160 * q + 32], ab[:, 0:32], 0.5,
                ab[:, 32:64], MULT, ADD)
            nc.scalar.activation(tch[:], xdt[:, 160 * q:160 * q + 32],
                                 TANH, scale=0.5)
            nc.vector.scalar_tensor_tensor(
                hview[:, :, j + 1, :],
                to.rearrange("p (k n) -> p k n", k=2, n=B), 1.0,
                tch[:].rearrange("p (k n) -> p k n", k=2, n=B), ADD, MULT)

        def block(blk, whh, wih, xsrc_d, dec_out=None, dbg_sb=None,
                  first=False):
            if first:
                stage8(blk, 0, xsrc_d)
                stage8(blk, 1, xsrc_d)
            cur = g_pool.tile([128, 512], F32, name="g_ps")
            for c in range(NCH):
                x_group_mm(c, 0, wih, cur)
            dec = None
            ostage = None
            if dec_out is not None:
                ostage = spool.tile([F, U * B], F32, name="ostage")
                dec = (ostage,)
            nxt = None
            for j in range(U):
                g = j // GRP
                if j % GRP == 0:
                    if g > 0:
                        cur = nxt
                    nxt = None
                    if g + 1 < NPG:
                        nxt = g_pool.tile([128, 512], F32, name="g_ps")
                # restage only after the x-MMs that read this ring half
                # have been emitted (they run at steps 8s..8s+3 for the
                # second psum-group): emit at 8s+4, consumed ~8 steps later.
                # s >= NSG prefetches the NEXT block (padded DRAM covers the
                # final block's overrun).
                if j % SG == 4:
                    stage8(blk, j // SG + 2, xsrc_d)
                step(j, whh, wih, cur, nxt, dec=dec, dbg_sb=dbg_sb)
            nc.vector.tensor_copy(hview[:, :, 0, :], hview[:, :, U, :])
            if dec_out is not None:
                nc.sync.dma_start(dec_out[:, bass.ts(blk, U * B)], ostage[:])

        def body():
            init_state()
            if static_loops:
                dbg_sb = None
                if dbg:
                    dt_sb = wpool.tile([128, 8 * 128], F32, name="dt_sb")
                    dg_sb = wpool.tile([128, 8 * 128], F32, name="dg_sb")
                    dbg_sb = (dt_sb, dg_sb)
                for blk in range(NB):
                    block(blk, whh_e, wih_e, xte_d,
                          dbg_sb=dbg_sb if blk == 0 else None,
                          first=(blk == 0))
                if dbg:
                    nc.sync.dma_start(dbg_t_d[:], dbg_sb[0][:])
                    nc.sync.dma_start(dbg_g_d[:], dbg_sb[1][:])
                    dh_sb = wpool.tile([128, 2 * HREG], F32, name="dh_sb")
                    nc.vector.tensor_copy(dh_sb[:], hhist[:])
                    nc.sync.dma_start(dbg_h_d[:], dh_sb[:])
                for blk in range(NB):
                    block(blk, whh_d, wih_d, xtd_d, dec_out=out_d,
                          first=(blk == 0))
            else:
                import concourse.mybir as _mb
                lkw = {}
                if loop_hints:
                    lkw["hint_engines"] = (
                        _mb.EngineType.PE, _mb.EngineType.Activation,
                        _mb.EngineType.DVE, _mb.EngineType.Pool,
                        _mb.EngineType.SP)
                if stag:
                    lkw["staggered_reset"] = True
                block(0, whh_e, wih_e, xte_d, first=True)
                with tc.For_i(1, NB // unroll, **lkw) as blk:
                    for u in range(unroll):
                        block(blk * unroll + u, whh_e, wih_e, xte_d)
                block(0, whh_d, wih_d, xtd_d, dec_out=out_d, first=True)
                with tc.For_i(1, NB // unroll, **lkw) as blk:
                    for u in range(unroll):
                        block(blk * unroll + u, whh_d, wih_d, xtd_d,
                              dec_out=out_d)

        if repeat == 1:
            body()
        else:
            with tc.For_i(0, repeat):
                body()
        if sink_d is not None:
            nc.sync.dma_start(sink_d[:], xdt[0:1, 0:4])

    nc.compile()
    return nc


def host_prep(ts_batch, W_ih_enc, W_hh_enc, b_enc, W_ih_dec, W_hh_dec, b_dec,
              W_out, b_out, T=1024):
    # gate rows pytorch [i f g o] -> kernel chunk order [f i g o];
    # row scale: g rows x2 (tanh(0.5*2g)=tanh(g)); h-consuming cols x0.5
    # (state h2 = 2h).
    perm = np.concatenate([np.arange(512, 768), np.arange(256, 512),
                           np.arange(0, 256), np.arange(768, 1024)])
    rs = np.ones(G, np.float32)
    rs[0:256] = 2.0  # g rows (first in permuted order g f i o)

    def prep_w(W_ih, W_hh, b):
        W_ih = np.asarray(W_ih, np.float32)[perm] * rs[:, None]   # [G, F]
        W_hh = np.asarray(W_hh, np.float32)[perm] * rs[:, None] * 0.5
        b = np.asarray(b, np.float32)[perm] * rs
        Wt = np.ascontiguousarray(W_hh.T)            # [H, G]
        whh_g = np.zeros((128, 16 * 128), np.float32)
        for c in range(NCH):
            for k in (0, 1):
                whh_g[:, (2 * c + k) * 128:(2 * c + k + 1) * 128] = \
                    Wt[128 * k:128 * (k + 1), 128 * c:128 * (c + 1)]
        wih_g = np.zeros((128, 8 * 128), np.float32)
        Wit = np.ascontiguousarray(W_ih.T)           # [F, G]
        for c in range(NCH):
            wih_g[0:F, c * 128:(c + 1) * 128] = Wit[:, 128 * c:128 * (c + 1)]
            wih_g[F, c * 128:(c + 1) * 128] = b[128 * c:128 * (c + 1)]
        return np.ascontiguousarray(wih_g), np.ascontiguousarray(whh_g)

    wih_e, whh_e = prep_w(W_ih_enc, W_hh_enc, b_enc)
    wih_d, whh_d = prep_w(W_ih_dec, W_hh_dec, b_dec)
    Wo = np.asarray(W_out, np.float32) * 0.5         # [F, H] (h2 = 2h)
    wout_g = np.zeros((128, 2 * F), np.float32)
    for k in (0, 1):
        wout_g[:, k * F:(k + 1) * F] = Wo.T[128 * k:128 * (k + 1), :]

    ts = np.asarray(ts_batch, np.float32)
    in_maps = []
    for d in range(N_CORES):
        tsl = ts[d * B:(d + 1) * B]                  # [16, T, F]
        XPAD = 2 * SG * B
        xte = np.zeros((F + 1, T * B + XPAD), np.float32)
        xte[:F, :T * B] = tsl.transpose(2, 1, 0).reshape(F, T * B)
        xte[F] = 1.0
        xtd = np.zeros((F + 1, T * B + XPAD), np.float32)
        xtd[:, :T * B] = np.ascontiguousarray(
            xte[:, :T * B].reshape(F + 1, T, B)[:, ::-1, :].reshape(
                F + 1, T * B))
        xtd[F] = 1.0
        in_maps.append({
            "xte": np.ascontiguousarray(xte), "xtd": xtd,
            "wih_e": wih_e, "whh_e": whh_e,
            "wih_d": wih_d, "whh_d": whh_d,
            "wout": wout_g,
        })
    return in_maps


def kernel(ts_batch, W_ih_enc, W_hh_enc, b_enc, W_ih_dec, W_hh_dec, b_dec,
           W_out, b_out):
    from concourse.bass_utils import run_bass_kernel_spmd

    T = ts_batch.shape[1]
    if "nc" not in _CACHE:
        _CACHE["nc"] = _build(T=T)
    nc = _CACHE["nc"]

    in_maps = host_prep(ts_batch, W_ih_enc, W_hh_enc, b_enc, W_ih_dec,
                        W_hh_dec, b_dec, W_out, b_out, T=T)
    res = run_bass_kernel_spmd(nc, in_maps, core_ids=list(range(N_CORES)))
    bo = np.asarray(b_out, np.float32)
    outs = []
    for r in res.results:
        arr = r["out"].reshape(F, T, B)       # decoder-order s, out^T
        outs.append(np.transpose(arr[:, ::-1, :], (2, 1, 0))
                    + bo[None, None, :])
    return np.ascontiguousarray(np.concatenate(outs, 0))
